# revision 25
# baseline (speedup 1.0000x reference)
"""Additive attention (Bahdanau) on 8 Trainium2 NeuronCores.

Reference computation (per batch b):
    Q[h]      = sum_e q[e] * Wa_w[h, e] + Wa_b[h]              q = last_decoder_output[b, 0]
    V[s, h]   = sum_e enc[s, e] * Ua_w[h, e] + Ua_b[h]
    energy[s] = sum_h v[h] * tanh(Q[h] + V[s, h])
    energy[s] = -1e10 where mask[s] == 0
    p         = softmax(energy)
    out[e]    = sum_s p[s] * enc[s, e]

Sharding: data-parallel over batch B=32 across 8 cores (4 batches/core).

Layout strategy (all layout work is host-side in make_in_maps, so the
device never transposes anything):
  * nat  [BPC, P, C, E]  bf16 -- enc in "natural" layout
    nat[b, p, c, e] = enc[b, c*128+p, e].  Used by the phase-2 weighted
    sum (s on partitions).  16.8 MB/core.
  * et8  [BPC, P, NJ, 2, S] fp8e4 -- (enc + c_b) TRANSPOSED (e on
    partitions), pre-laid-out for the DoubleRow V matmul:
    et8[b, p, jp, j, s] = enc[b, s, 128*(2jp+j)+p] + c_b[128*(2jp+j)+p].
    8.4 MB/core.  c_b = Ua^T (Ua Ua^T)^{-1} g_b is the minimum-norm
    preimage of the per-batch additive constant g_b = Wa q_b + Wa_b +
    Ua_b, so (enc + c_b) @ Ua^T = V + g_b exactly -- the bias needs no
    on-chip work at all.
  * uat8 [P, NJ, 2, H] fp8e4 -- Ua^T pre-scaled by 256 (clears the fp8
    subnormal range; tanh applies scale=1/256 on the way out of PSUM).
  * vcol [P, HK] bf16 -- v with h on partitions: vcol[p, hk] = v[128*hk+p].
  * mb   [P, BPC, C] f32 -- mask additive bias {0, -1e10} in energy layout.

Per-core dataflow (per batch, per super-column sc of 512 s-values):
  V^T+g = uat8^T @ et8 on PE (fp8 DoubleRow, 512-col moving operand,
  out [128h, 512s] per hk half in PSUM); tanh (ACT, scale=1/256) ->
  SBUF bf16 [128, HK, 512]; energy columns on PE: tiny matmuls
  en[:, c] += th[:, hk, c-block]^T @ vcol[:, hk] accumulate into a
  PSUM energy tile [128, C] in softmax layout (s%128 partitions).
  softmax: masked bias add (DVE, reads PSUM), exp (ACT, accumulates
  row sums), Z via gpsimd partition_all_reduce, reciprocal.  No
  max-subtraction needed: |energy| <= sum|v| ~ 0.25 so exp never
  overflows, and masked entries are exactly exp(-1e10) = 0.
  phase 2: out = sum_s p~[s] * enc[s, :] as 32 accumulating matmuls with
  p~ columns stationary (bf16, DoublePixel), then scale by 1/Z.

Scheduling: all large DMAs ride the two HWDGE rings (sync + scalar) and
are emitted in need-order ahead of the compute that consumes them --
the scalar ring's issues all precede the tanh stream so they never
queue behind ACT work.  The energy matmuls run with a lag of 2
super-columns behind the V matmuls so the PE never stalls waiting for
tanh; batch b-1's phase-2 matmuls are spread between batch b's V
groups so the PE stream stays dense (HAM stays warm).
"""

import sys

if "/opt/trn_rl_repo" not in sys.path:
    sys.path.insert(0, "/opt/trn_rl_repo")

import numpy as np

import concourse.bass as bass  # noqa: F401  (engine types resolve through nc)
import concourse.mybir as mybir
import concourse.tile as tile
from concourse import bacc
from concourse.bass_utils import run_bass_kernel_spmd

F32 = mybir.dt.float32
BF16 = mybir.dt.bfloat16
FP8 = mybir.dt.float8e4
I32 = mybir.dt.int32
AF = mybir.ActivationFunctionType
ALU = mybir.AluOpType

N_CORES = 8
P = 128  # partitions
UA_SCALE = 256.0  # fp8 pre-scale on Ua^T (undone by tanh's scale=1/256)
SC = 512  # s-values per V-matmul super-column


def build_kernel(BPC=4, S=4096, E=512, H=256, use_dp=True):
    """Build the per-core Bass graph. All 8 cores run the same program."""
    C = S // P        # softmax / phase-2 columns (s = c*128 + p)
    EK = E // P       # e-chunks of 128
    NJ = EK // 2      # DoubleRow e-chunk pairs
    HK = H // P       # h-chunks of 128
    NSC = S // SC     # super-columns per batch
    CPS = SC // P     # energy columns per super-column

    nc = bacc.Bacc(None, target_bir_lowering=False)

    natp_d = nc.declare_dram_parameter("natp", [BPC, P, C, E], BF16, isOutput=False)
    et8_d = nc.declare_dram_parameter("et8", [BPC, P, NJ, 2, S], FP8, isOutput=False)
    uat8_d = nc.declare_dram_parameter("uat8", [P, NJ, 2, H], FP8, isOutput=False)
    vcol_d = nc.declare_dram_parameter("vcol", [P, HK], BF16, isOutput=False)
    mb_d = nc.declare_dram_parameter("mb", [P, BPC, C], F32, isOutput=False)
    out_d = nc.declare_dram_parameter("out", [BPC, E], F32, isOutput=True)

    with tile.TileContext(nc) as tc:
        with (
            tc.tile_pool(name="const", bufs=1) as const,
            tc.tile_pool(name="nat", bufs=3) as natp,
            tc.tile_pool(name="et8", bufs=4) as etp,
            tc.tile_pool(name="tanh", bufs=4) as tanhp,
            tc.tile_pool(name="sm", bufs=4) as smp,
            tc.tile_pool(name="v_ps", bufs=2, space="PSUM") as vpp,
            tc.tile_pool(name="en_ps", bufs=2, space="PSUM") as enp,
            tc.tile_pool(name="w_ps", bufs=2, space="PSUM") as wpp,
        ):
            nat_t = {}
            et_t = {}

            def dma_et8(b, eng, eng2=None):
                et_t[b] = etp.tile([P, NJ, 2, S], FP8, tag="et8", name=f"et{b}")
                if eng2 is None:
                    eng.dma_start(out=et_t[b], in_=et8_d[b])
                else:  # race the two halves on both rings
                    h = S // 2
                    eng.dma_start(out=et_t[b][:, :, :, :h], in_=et8_d[b, :, :, :, :h])
                    eng2.dma_start(out=et_t[b][:, :, :, h:], in_=et8_d[b, :, :, :, h:])

            def dma_nat(b, half, eng):
                # tile created at first touch; nat3 (buffer reuse with
                # bufs=3) must be created only after phase2(0) is emitted
                if b not in nat_t:
                    nat_t[b] = natp.tile([P, C, E], BF16, tag="nat", name=f"nat{b}")
                sl = slice(half * C // 2, (half + 1) * C // 2)
                eng.dma_start(out=nat_t[b][:, sl, :], in_=natp_d[b, :, sl, :])

            # ---- prologue ----
            # Two HWDGE rings (sync + scalar); SWDGE mixes badly with HWDGE
            # (arbitration starves the HWDGE ring), so everything is HWDGE.
            # Each ring completes in enqueue order, so enqueue order per
            # ring must match consumption order:
            #   sync:   consts, et8[0..3], nat3h0, nat3h1, outs
            #   scalar: nat0h0, nat0h1, nat1h0 | b1: nat1h1, nat2h0 | b2: nat2h1
            # The scalar engine also runs the tanh stream: it gets at most 3
            # prologue issues (within HWDGE ring credit, so they don't
            # block) and 2-issue top-ups at batch boundaries.  et8 has
            # bufs=4 so no et8 DMA carries a buffer-reuse dependency; nat3
            # (bufs=3) reuses nat0's buffer and its issue blocks the sync
            # engine until phase2(0) drains -- harmless, sync only issues
            # DMAs.
            # Consts ride SWDGE (gpsimd): tiny, drain instantly, and SWDGE
            # also outranks the sync ring so they arrive immediately.
            uat8_sb = const.tile([P, NJ, 2, H], FP8)
            nc.gpsimd.dma_start(out=uat8_sb, in_=uat8_d[:, :, :, :])
            vcol_sb = const.tile([P, HK], BF16)
            nc.gpsimd.dma_start(out=vcol_sb, in_=vcol_d[:, :])
            mb_sb = const.tile([P, BPC, C], F32)
            nc.gpsimd.dma_start(out=mb_sb, in_=mb_d[:, :, :])

            # The scalar HWDGE ring outranks the sync ring in SDMA
            # arbitration, so the critical need-ordered stream rides it:
            # 3 issues now (within ring credit, non-blocking), topped up 2
            # per batch boundary between tanh batches.  The sync ring
            # carries only the late tail + outputs.
            dma_et8(0, nc.scalar)
            dma_et8(1, nc.scalar)
            dma_nat(0, 0, nc.scalar)
            dma_nat(2, 1, nc.sync)

            # HAM warmup: keep the PE busy while et8[0] streams in
            warm_sb = const.tile([P, 2, H], F32)
            nc.vector.memset(warm_sb, 0.0)
            for _ in range(3):
                w_ps0 = wpp.tile([1, E], F32, tag="w_ps", name="warmup_ps")
                nc.tensor.matmul(
                    w_ps0,
                    lhsT=warm_sb[:, 0, 0:1],
                    rhs=warm_sb[:, :, :],
                    start=True,
                    stop=True,
                )

            out_sb = const.tile([1, BPC, E], F32)

            en_t = {}
            th_t = {}
            pt_t = {}
            rz_t = {}
            wps_t = {}

            def emit_vmm(b, sc):
                # V^T + g for s in [sc*512, (sc+1)*512): PSUM [128h, HK, 512s]
                v_ps = vpp.tile([P, HK, SC], F32, tag="v_ps")
                ssl = slice(SC * sc, SC * (sc + 1))
                for hk in range(HK):
                    for jp in range(NJ):
                        nc.tensor.matmul(
                            v_ps[:, hk, :],
                            lhsT=uat8_sb[:, jp, :, P * hk : P * (hk + 1)],
                            rhs=et_t[b][:, jp, :, ssl],
                            perf_mode=mybir.MatmulPerfMode.DoubleRow,
                            start=(jp == 0),
                            stop=(jp == NJ - 1),
                        )
                th = tanhp.tile([P, HK, SC], BF16, tag="tanh")
                nc.scalar.activation(th, v_ps, AF.Tanh, scale=1.0 / UA_SCALE)
                th_t[(b, sc)] = th

            def emit_energy(b, sc):
                # en[:, c] += th[:, hk, c-block]^T @ vcol[:, hk]
                if sc == 0:
                    en_t[b] = enp.tile([P, C], F32, tag="en_ps", name=f"en{b}")
                th = th_t.pop((b, sc))
                for c4 in range(CPS):
                    c = CPS * sc + c4
                    for hk in range(HK):
                        nc.tensor.matmul(
                            en_t[b][:, c : c + 1],
                            lhsT=th[:, hk, P * c4 : P * (c4 + 1)],
                            rhs=vcol_sb[:, hk : hk + 1],
                            start=(hk == 0),
                            stop=(hk == HK - 1),
                        )

            def emit_softmax(b):
                em = smp.tile([P, C], F32, tag="em")
                nc.vector.tensor_add(em, en_t[b], mb_sb[:, b, :])
                pt_t[b] = smp.tile([P, C], BF16, tag="pt", name=f"pt{b}")
                zrow = smp.tile([P, 1], F32, tag="zrow")
                nc.scalar.activation(pt_t[b], em, AF.Exp, accum_out=zrow)
                zred = smp.tile([P, 1], F32, tag="zred")
                nc.gpsimd.partition_all_reduce(
                    zred, zrow, channels=P, reduce_op=bass.bass_isa.ReduceOp.add
                )
                rz_t[b] = smp.tile([1, 1], F32, tag="rz", name=f"rz{b}")
                nc.vector.reciprocal(rz_t[b], zred[0:1, 0:1])
                wps_t[b] = wpp.tile([1, E], F32, tag="w_ps", name=f"wps{b}")

            def emit_wmms(b, c):
                w_ps = wps_t[b]
                nc.tensor.matmul(
                    w_ps,
                    lhsT=pt_t[b][:, c : c + 1],
                    rhs=nat_t[b][:, c, :],
                    perf_mode=(mybir.MatmulPerfMode.DoublePixel if use_dp else None),
                    start=(c == 0),
                    stop=(c == C - 1),
                )
                if c == C - 1:
                    nc.vector.tensor_scalar(
                        out=out_sb[:, b, :],
                        in0=w_ps,
                        scalar1=rz_t[b][0:1, 0:1],
                        scalar2=None,
                        op0=ALU.mult,
                    )

            # ---- software-pipelined batch loop ----
            LAG = 2  # energy trails V by LAG super-columns (tanh drains)
            wq = []  # pending weighted-sum matmuls (b, c)
            for b in range(BPC):
                if b == 1:
                    dma_et8(2, nc.scalar)
                    dma_nat(0, 1, nc.scalar)
                if b == 2:
                    dma_et8(3, nc.scalar)
                    dma_nat(1, 0, nc.scalar)
                if b == 3:
                    dma_nat(1, 1, nc.scalar)
                    dma_nat(2, 0, nc.scalar)
                    dma_nat(3, 0, nc.sync)
                    dma_nat(3, 1, nc.sync)
                for sc in range(NSC):
                    emit_vmm(b, sc)
                    if sc >= LAG:
                        emit_energy(b, sc - LAG)
                    # spread weighted-sum matmuls between V-matmul groups;
                    # 2/group spreads each batch's 32 over TWO batches so
                    # nat[b]'s two halves are needed a full batch apart
                    for _ in range(2):
                        if wq:
                            emit_wmms(*wq.pop(0))
                for sc in range(NSC - LAG, NSC):
                    emit_energy(b, sc)
                emit_softmax(b)
                wq.extend((b, c) for c in range(C))
            while wq:
                emit_wmms(*wq.pop(0))

            for b in range(BPC):
                nc.sync.dma_start(out=out_d[b : b + 1, :], in_=out_sb[:, b, :])

    nc.finalize()
    return nc


_CACHE = {}


def _get_kernel(key):
    if key not in _CACHE:
        _CACHE[key] = build_kernel(*key)
    return _CACHE[key]


def make_in_maps(enc, ldo, mask, v, Ua_w, Ua_b, Wa_w, Wa_b, bpc, n_cores):
    """Shard + lay out host-side. enc: [B,S,2H] f32, mask: [B,S] i32."""
    import ml_dtypes

    B, S, E = enc.shape
    H = Wa_w.shape[0]
    C = S // P
    EK = E // P
    NJ = EK // 2
    HK = H // P
    BF = ml_dtypes.bfloat16
    F8 = ml_dtypes.float8_e4m3

    ua = Ua_w.astype(np.float64)  # [H, E]
    # g_b = Wa_w @ q_b + Wa_b + Ua_b; c_b = Ua^T (Ua Ua^T)^{-1} g_b so that
    # (enc + c_b) @ Ua^T = V + g_b exactly (minimum-norm preimage).
    g = (
        ldo[:, 0, :].astype(np.float64) @ Wa_w.astype(np.float64).T
        + Wa_b.astype(np.float64)
        + Ua_b.astype(np.float64)
    )  # [B, H]
    cmat = ua.T @ np.linalg.solve(ua @ ua.T, g.T)  # [E, B]

    enc = np.ascontiguousarray(enc.astype(np.float32))
    # nat[b, p, c, e] = enc[b, c*128+p, e]
    nat_all = np.ascontiguousarray(
        enc.reshape(B, C, P, E).transpose(0, 2, 1, 3).astype(BF)
    )  # [B, P, C, E]
    # et8[b, p, jp, j, s] = enc[b, s, 128*(2jp+j)+p] + c_b[128*(2jp+j)+p]
    encc = enc + cmat.T.astype(np.float32)[:, None, :]  # [B, S, E]
    et8_all = np.ascontiguousarray(
        encc.reshape(B, S, NJ, 2, P).transpose(0, 4, 2, 3, 1).astype(F8)
    )  # [B, P, NJ, 2, S]

    # uat8[p, jp, j, h] = Ua_w[h, 128*(2*jp+j)+p] * UA_SCALE
    uat = np.ascontiguousarray(Ua_w.T.astype(np.float32))  # [E, H]
    uat8 = np.ascontiguousarray(
        (uat * UA_SCALE).reshape(NJ, 2, P, H).transpose(2, 0, 1, 3).astype(F8)
    )  # [P, NJ, 2, H]

    # vcol[p, hk] = v[128*hk + p]
    vcol = np.ascontiguousarray(
        v.astype(np.float32).reshape(HK, P).T.astype(BF)
    )  # [P, HK]

    # mb[p, b, c] = 0 if mask[b, c*128+p] else -1e10
    mb = np.ascontiguousarray(
        np.where(mask.astype(np.int32) == 0, np.float32(-1e10), np.float32(0.0))
        .reshape(B, C, P)
        .transpose(2, 0, 1)
    )  # [P, B, C]

    in_maps = []
    for c in range(n_cores):
        lo, hi = c * bpc, (c + 1) * bpc
        in_maps.append(
            {
                "natp": nat_all[lo:hi],
                "et8": et8_all[lo:hi],
                "uat8": uat8,
                "vcol": vcol,
                "mb": np.ascontiguousarray(mb[:, lo:hi, :]),
            }
        )
    return in_maps


def kernel(
    encoder_output,
    last_decoder_output,
    src_attention_mask,
    v,
    Ua_w,
    Ua_b,
    Wa_w,
    Wa_b,
):
    enc = np.asarray(encoder_output)
    B, S, E = enc.shape
    bpc = B // N_CORES
    in_maps = make_in_maps(
        enc,
        np.asarray(last_decoder_output),
        np.asarray(src_attention_mask),
        np.asarray(v),
        np.asarray(Ua_w),
        np.asarray(Ua_b),
        np.asarray(Wa_w),
        np.asarray(Wa_b),
        bpc,
        N_CORES,
    )
    nc = _get_kernel((bpc, S, E, Wa_w.shape[0]))
    res = run_bass_kernel_spmd(nc, in_maps, core_ids=list(range(N_CORES)))
    out = np.concatenate([res.results[i]["out"] for i in range(N_CORES)], axis=0)
    return out[:, None, :].astype(np.float32)
